# revision 1
# baseline (speedup 1.0000x reference)
"""Causal multi-head self-attention (RoPE) Trainium2 Bass kernel.

Problem: B=4, T=2048, d_model=1024, 16 heads, d_k=64, causal, RoPE,
fp32 I/O.  Sharding: batch (4-way) x head-group (2-way) over 8 cores.
Core c handles batch c//2 and heads [8*(c%2) .. 8*(c%2)+8).

Everything on device runs in the "transposed" domain to avoid on-chip
transposes entirely:
  QT/KT:  [head_dim, T]   (head dim on partitions)
  V:      [T, head_dim]   (k positions on partitions)
  scores: S^T [k, q] = KT_tile^T @ QT  (per head)
  E = exp(S^T/8), causally masked via gpsimd affine_select
  PV: H^T[d, q] = [V|ones]^T @ E  -> heads rows 0:64 + sums row 64
  normalize via ones-matmul broadcast of sums + reciprocal + TT multiply
  out-proj: y[t, e] = H^T_tile^T @ WoT, accumulated over head pairs

RoPE is applied as rot = cos (.) pre + sin (.) (P2 @ pre), where the
head dim has been host-permuted to rotate-half layout and P2 is the
fixed swap-negate permutation folded into a 128x128 matmul.

Matmuls use float32r (full-rate fp32 mode, ~1e-4 relative precision).
The k-loop and projection loops are software-pipelined by one stage so
the PE never stalls on ACT (exp / psum evacuation) latency.
"""

import numpy as np
from contextlib import ExitStack

import concourse.bass as bass
from concourse import bacc
import concourse.tile as tile
import concourse.mybir as mybir
import ml_dtypes
from concourse.bass_utils import run_bass_kernel_spmd

F32 = mybir.dt.float32
F32R = mybir.dt.float32r
BF16 = mybir.dt.bfloat16
AF = mybir.ActivationFunctionType
ALU = mybir.AluOpType

D_MODEL = 1024
NUM_HEADS = 16
THETA = 10000.0
B, T = 4, 2048
N_CORES = 8
PAIRS = 4             # head pairs per core (8 heads)
QC = 512              # q-chunk width
NQC = T // QC
KT = 128              # k-tile height
NKT = T // KT
XC = 256              # xt streaming chunk width (phase A)
NXC = T // XC
DMT = D_MODEL // 128  # 8 d_model k-tiles

_CACHE = {}


def _build_nc():
    nc = bacc.Bacc(None, target_bir_lowering=False)

    xt = nc.dram_tensor("xt", [D_MODEL, T], F32R, kind="ExternalInput")
    wq = nc.dram_tensor("wq", [D_MODEL, 512], F32R, kind="ExternalInput")
    wk = nc.dram_tensor("wk", [D_MODEL, 512], F32R, kind="ExternalInput")
    wv = nc.dram_tensor("wv", [D_MODEL, 512], F32R, kind="ExternalInput")
    wo = nc.dram_tensor("wo", [512, D_MODEL], F32R, kind="ExternalInput")
    cosd = nc.dram_tensor("cos", [128, T], F32, kind="ExternalInput")
    sind = nc.dram_tensor("sin", [128, T], F32, kind="ExternalInput")
    p2t = nc.dram_tensor("p2t", [128, 128], F32R, kind="ExternalInput")
    negi = nc.dram_tensor("negi", [128, 128], BF16, kind="ExternalInput")
    triu = nc.dram_tensor("triu", [128, 128], BF16, kind="ExternalInput")
    y = nc.dram_tensor("y", [T, D_MODEL], F32, kind="ExternalOutput")

    with tile.TileContext(nc) as tc, ExitStack() as ctx:
        constp = ctx.enter_context(tc.tile_pool(name="const", bufs=1))
        qkv_stack = ExitStack()
        qkp = qkv_stack.enter_context(tc.tile_pool(name="qk", bufs=1))
        vp = qkv_stack.enter_context(tc.tile_pool(name="v", bufs=1))

        cos_sb = constp.tile([128, T], F32)
        sin_sb = constp.tile([128, T], F32)
        p2_sb = constp.tile([128, 128], F32R)
        negi_sb = constp.tile([128, 128], BF16)
        triu_sb = constp.tile([128, 128], BF16)
        ones_sb = constp.tile([128, 64], F32R)
        nc.vector.memset(ones_sb.bitcast(F32), 1.0)

        qt_t = [qkp.tile([128, T], F32R, name=f"qt{p}", tag=f"qt{p}")
                for p in range(PAIRS)]
        kt_t = [qkp.tile([128, T], F32R, name=f"kt{p}", tag=f"kt{p}")
                for p in range(PAIRS)]
        # V with interleaved ones cols: per k-tile, per pair:
        # [V_h0(64) | 1 | V_h1(64) | 1] = 130 cols
        v_sb = vp.tile([128, NKT, PAIRS * 130], F32R)
        v5 = v_sb.rearrange("p t (pr x) -> p t pr x", pr=PAIRS)
        v6 = v5.rearrange("p t pr (hl c) -> p t pr hl c", hl=2)
        nc.vector.memset(v6[:, :, :, :, 64:65].bitcast(F32), 1.0)

        # ---------------- Phase A: projections + RoPE ----------------
        with tc.tile_pool(name="wqkv", bufs=1) as wp, \
             tc.tile_pool(name="xtp", bufs=2) as xtp, \
             tc.tile_pool(name="sA", bufs=3) as sA, \
             tc.tile_pool(name="psA", bufs=2, space="PSUM") as psA, \
             tc.tile_pool(name="psV", bufs=2, space="PSUM") as psV:
            wq_sb = wp.tile([128, DMT, 512], F32R)
            wk_sb = wp.tile([128, DMT, 512], F32R)
            wv_sb = wp.tile([128, DMT, 512], F32R)
            xt_r = xt.rearrange("(a p) t -> p a t", p=128)
            wq_r = wq.rearrange("(a p) m -> p a m", p=128)
            wk_r = wk.rearrange("(a p) m -> p a m", p=128)
            wv_r = wv.rearrange("(a p) m -> p a m", p=128)
            # priority order: wq + first x chunk feed the very first
            # matmul group; everything else can trickle in behind them.
            for dmt in range(DMT):
                nc.sync.dma_start(wq_sb[:, dmt, :], wq_r[:, dmt, :])
            pre_x = {}
            xs0 = xtp.tile([128, DMT, XC], F32R, tag="xt", name="xpre0")
            nc.sync.dma_start(xs0, xt_r[:, :, 0:XC])
            pre_x[0] = xs0
            for dmt in range(DMT):
                nc.sync.dma_start(wk_sb[:, dmt, :], wk_r[:, dmt, :])
            xs1 = xtp.tile([128, DMT, XC], F32R, tag="xt", name="xpre1")
            nc.sync.dma_start(xs1, xt_r[:, :, XC:2 * XC])
            pre_x[1] = xs1
            nc.sync.dma_start(p2_sb, p2t[:])
            nc.sync.dma_start(cos_sb, cosd[:])
            nc.sync.dma_start(sin_sb, sind[:])
            nc.sync.dma_start(negi_sb, negi[:])
            nc.sync.dma_start(triu_sb, triu[:])
            for dmt in range(DMT):
                nc.sync.dma_start(wv_sb[:, dmt, :], wv_r[:, dmt, :])

            def emit_v(c, x_sb):
                # V for the two k-tiles of chunk c (deferred one chunk so
                # the wv DMA has time to land)
                for vt in range(2):
                    kti = c * 2 + vt
                    ps_v = psV.tile([128, 512], F32, tag="v", name="ps_v")
                    for dmt in range(DMT):
                        nc.tensor.matmul(
                            ps_v, x_sb[:, dmt, vt * 128:(vt + 1) * 128],
                            wv_sb[:, dmt, :],
                            start=(dmt == 0), stop=(dmt == DMT - 1))
                    src = ps_v.rearrange("p (pr hl c) -> p pr hl c",
                                         pr=PAIRS, hl=2)
                    nc.scalar.copy(v6[:, kti, :, :, 0:64], src)

            def rope_stage(st):
                pre, dst, cs = st
                ps_a = psA.tile([128, XC], F32, tag="alt", name="ps_a")
                nc.tensor.matmul(ps_a, p2_sb, pre, start=True, stop=True)
                t1 = sA.tile([128, XC], F32, tag="t1", name="t1")
                nc.vector.tensor_mul(t1, sin_sb[:, cs], ps_a)
                t2 = sA.tile([128, XC], F32, tag="t2", name="t2")
                nc.gpsimd.tensor_mul(t2, cos_sb[:, cs], pre)
                nc.vector.tensor_add(dst[:, cs], t1, t2)

            for c in range(NXC):
                cs = slice(c * XC, (c + 1) * XC)
                if c in pre_x:
                    x_sb = pre_x.pop(c)
                else:
                    x_sb = xtp.tile([128, DMT, XC], F32R, tag="xt",
                                    name="x_sb")
                    nc.sync.dma_start(x_sb, xt_r[:, :, cs])

                pend = None
                if c > 0:
                    emit_v(c - 1, pend_v)
                for (w_sb, dsts) in ((wq_sb, qt_t), (wk_sb, kt_t)):
                    for p in range(PAIRS):
                        ps = psA.tile([128, XC], F32, tag="proj", name="ps")
                        for dmt in range(DMT):
                            nc.tensor.matmul(
                                ps, w_sb[:, dmt, p * 128:(p + 1) * 128],
                                x_sb[:, dmt, :],
                                start=(dmt == 0), stop=(dmt == DMT - 1))
                        pre = sA.tile([128, XC], F32R, tag="pre", name="pre")
                        nc.scalar.copy(pre, ps)
                        if pend is not None:
                            rope_stage(pend)
                        pend = (pre, dsts[p], cs)

                rope_stage(pend)
                pend_v = x_sb
            emit_v(NXC - 1, pend_v)

        # ---------------- Phase B: attention ----------------
        hp = ctx.enter_context(tc.tile_pool(name="hp", bufs=1, side="right"))
        h_t = [hp.tile([128, T], F32R, name=f"h{p}", tag=f"h{p}")
               for p in range(PAIRS)]
        with tc.tile_pool(name="ep", bufs=4) as ep, \
             tc.tile_pool(name="h1p", bufs=2) as h1p, \
             tc.tile_pool(name="sB", bufs=2) as sB, \
             tc.tile_pool(name="psS", bufs=2, space="PSUM") as psS, \
             tc.tile_pool(name="psH", bufs=2, space="PSUM") as psH:

            def emit_pv(st):
                p, ps_h, e_sb, c0, k, last = st
                nc.tensor.matmul(ps_h[0:65, 0, c0:],
                                 v_sb[:, k, 130 * p:130 * p + 65],
                                 e_sb[:, 0, c0:],
                                 start=(k == 0), stop=last)
                nc.tensor.matmul(ps_h[0:65, 1, c0:],
                                 v_sb[:, k, 130 * p + 65:130 * p + 130],
                                 e_sb[:, 1, c0:],
                                 start=(k == 0), stop=last)

            def emit_norm(st):
                p, ps_h, qc, h1 = st
                s2 = sB.tile([128, 2, 512], F32R, tag="s2", name="s2")
                nc.vector.tensor_copy(s2[64:65, :, :], ps_h[64:65, :, :])
                ps_r = psS.tile([64, 2, 512], F32, tag="s", name="ps_r")
                nc.tensor.matmul(ps_r[0:64, 0, :], ones_sb[64:65, :],
                                 s2[64:65, 0, :], start=True, stop=True)
                nc.tensor.matmul(ps_r[0:64, 1, :], ones_sb[64:65, :],
                                 s2[64:65, 1, :], start=True, stop=True)
                psS.tile([128, 2, 512], F32, tag="s", name="ps_dummy")
                r_sb = sB.tile([64, 2, 512], F32, tag="r", name="r_sb")
                nc.vector.reciprocal_approx_fast(out=r_sb[0:64, 0, :],
                                                 in_=ps_r[0:64, 0, :])
                nc.vector.reciprocal_approx_fast(out=r_sb[0:64, 1, :],
                                                 in_=ps_r[0:64, 1, :])
                qs = slice(qc * QC, (qc + 1) * QC)
                nc.vector.tensor_mul(h_t[p][0:64, qs], ps_h[0:64, 0, :],
                                     r_sb[0:64, 0, :])
                nc.vector.tensor_mul(h1[0:64, :], ps_h[0:64, 1, :],
                                     r_sb[0:64, 1, :])
                # odd head rows into partitions 64:128 of the pair tile
                nc.sync.dma_start(h_t[p][64:128, qs], h1[0:64, :])

            pend_pv = []
            pend_norm = None
            for p in range(PAIRS):
                qt, kt = qt_t[p], kt_t[p]
                for qc in range(NQC):
                    nk = 4 * (qc + 1)
                    ps_h = psH.tile([128, 2, 512], F32, tag="pv", name="ps_h")
                    h1 = h1p.tile([64, QC], F32R, tag="h1", name="h1")
                    for k in range(nk):
                        m = k - 4 * qc
                        c0 = 128 * m if m >= 0 else 0
                        qs = slice(qc * QC + c0, (qc + 1) * QC)
                        ks = slice(k * KT, (k + 1) * KT)
                        ps_s = psS.tile([128, 2, 512], F32, tag="s",
                                        name="ps_s")
                        diag = m >= 0
                        if diag:
                            # preload -1e30 strictly-upper-tri (k>q) so the
                            # scores accumulate on top and exp yields 0
                            nc.tensor.matmul(ps_s[:, 0, c0:c0 + 128],
                                             negi_sb, triu_sb,
                                             start=True, stop=False)
                            nc.tensor.matmul(ps_s[:, 1, c0:c0 + 128],
                                             negi_sb, triu_sb,
                                             start=True, stop=False)
                        nc.tensor.matmul(ps_s[:, 0, c0:], kt[0:64, ks],
                                         qt[0:64, qs], start=not diag,
                                         stop=True)
                        nc.tensor.matmul(ps_s[:, 1, c0:], kt[64:128, ks],
                                         qt[64:128, qs], start=not diag,
                                         stop=True)
                        e_sb = ep.tile([128, 2, 512], F32R, tag="e",
                                       name="e_sb")
                        nc.scalar.activation(e_sb[:, :, c0:], ps_s[:, :, c0:],
                                             AF.Exp, scale=0.125)
                        if len(pend_pv) >= 2:
                            emit_pv(pend_pv.pop(0))
                        pend_pv.append((p, ps_h, e_sb, c0, k, k == nk - 1))
                        # fire the deferred norm only once the previous
                        # q-chunk's last PV has drained out of pend_pv
                        if pend_norm is not None and k >= 2:
                            emit_norm(pend_norm)
                            pend_norm = None
                    pend_norm = (p, ps_h, qc, h1)
            for st in pend_pv:
                emit_pv(st)
            emit_norm(pend_norm)

        qkv_stack.close()

        # ---------------- Phase C: output projection ----------------
        with tc.tile_pool(name="wop", bufs=1) as wop, \
             tc.tile_pool(name="yst", bufs=3) as yst, \
             tc.tile_pool(name="psY", bufs=4, space="PSUM") as psY:
            wo_sb = wop.tile([128, PAIRS, D_MODEL], F32R)
            nc.sync.dma_start(
                wo_sb, wo.rearrange("(a p) m -> p a m", p=128))
            for tt in range(T // 128):
                ts_ = slice(tt * 128, (tt + 1) * 128)
                for ec in range(2):
                    es = slice(ec * 512, (ec + 1) * 512)
                    ps_y = psY.tile([128, 512], F32, tag="y", name="ps_y")
                    for p in range(PAIRS):
                        nc.tensor.matmul(ps_y, h_t[p][:, ts_],
                                         wo_sb[:, p, es],
                                         start=(p == 0),
                                         stop=(p == PAIRS - 1))
                    y_sb = yst.tile([128, 512], F32, tag="y", name="y_sb")
                    nc.scalar.copy(y_sb, ps_y)
                    nc.sync.dma_start(y[ts_, es], y_sb)

    nc.compile()
    return nc


def _host_prep(in_features, token_positions, Wq, Wk, Wv, Wo):
    """Shard + pre-transpose inputs for the 8 cores."""
    x = np.asarray(in_features, dtype=np.float32)
    pos = np.asarray(token_positions)
    Wq = np.asarray(Wq, dtype=np.float32)
    Wk = np.asarray(Wk, dtype=np.float32)
    Wv = np.asarray(Wv, dtype=np.float32)
    Wo = np.asarray(Wo, dtype=np.float32)

    # rotate-half permutation of each head's 64 dims: evens then odds
    perm = np.concatenate([np.arange(0, 64, 2), np.arange(1, 64, 2)])
    full_perm = (np.arange(NUM_HEADS)[:, None] * 64 + perm[None, :]).reshape(-1)
    Wq_p = Wq[full_perm, :]   # permute output rows (head dims)
    Wk_p = Wk[full_perm, :]

    # P2: alt = P2 @ pre (per 64-block: alt[i] = -pre[32+i], alt[32+i]=pre[i])
    p2 = np.zeros((128, 128), np.float32)
    for blk in (0, 64):
        for i in range(32):
            p2[blk + i, blk + 32 + i] = -1.0
            p2[blk + 32 + i, blk + i] = 1.0
    p2t = np.ascontiguousarray(p2.T)

    inv_freq = 1.0 / (THETA ** (np.arange(32, dtype=np.float64) * 2.0 / 64))

    negi = (np.eye(128, dtype=np.float32) * -1e30).astype(ml_dtypes.bfloat16)
    # triu[p, j] = 1 where k (=p) > q (=j): strictly lower-left in [k, q]
    jj = np.arange(128)
    triu = (jj[None, :] < jj[:, None]).astype(ml_dtypes.bfloat16)

    in_maps = []
    for core in range(N_CORES):
        b = core // 2
        g = core % 2
        hs = slice(g * 512, (g + 1) * 512)   # head-dim slice of d_model

        ang = pos[b].astype(np.float64)[None, :] * inv_freq[:, None]  # [32,T]
        cos64 = np.cos(ang).astype(np.float32)
        sin64 = np.sin(ang).astype(np.float32)
        cos128 = np.tile(np.concatenate([cos64, cos64], 0), (2, 1))   # [128,T]
        sin128 = np.tile(np.concatenate([sin64, sin64], 0), (2, 1))

        in_maps.append({
            "xt": np.ascontiguousarray(x[b].T),
            "wq": np.ascontiguousarray(Wq_p[hs, :].T),
            "wk": np.ascontiguousarray(Wk_p[hs, :].T),
            "wv": np.ascontiguousarray(Wv[hs, :].T),
            "wo": np.ascontiguousarray(Wo[:, hs].T),
            "cos": np.ascontiguousarray(cos128),
            "sin": np.ascontiguousarray(sin128),
            "p2t": p2t,
            "negi": negi,
            "triu": triu,
        })
    return in_maps


def kernel(**inputs):
    if "nc" not in _CACHE:
        _CACHE["nc"] = _build_nc()
    nc = _CACHE["nc"]
    in_maps = _host_prep(**inputs)
    res = run_bass_kernel_spmd(nc, in_maps, core_ids=list(range(N_CORES)))
    out = np.zeros((B, T, D_MODEL), np.float32)
    for core in range(N_CORES):
        out[core // 2] += res.results[core]["y"]
    return out



# revision 2
# speedup vs baseline: 1.0765x; 1.0765x over previous
"""Causal multi-head self-attention (RoPE) Trainium2 Bass kernel.

Problem: B=4, T=2048, d_model=1024, 16 heads, d_k=64, causal, RoPE,
fp32 I/O.  Sharding: batch (4-way) x head-group (2-way) over 8 cores.
Core c handles batch c//2 and heads [8*(c%2) .. 8*(c%2)+8).

Everything on device runs in the "transposed" domain to avoid on-chip
transposes entirely:
  QT/KT:  [head_dim, T]   (head dim on partitions), bf16
  V:      [T, head_dim]   (k positions on partitions), bf16 + ones col
  scores: S^T [k, q] = KT_tile^T @ QT  (per head), fp32 PSUM
  E = exp(S^T/8) in bf16; causal masking of the diagonal blocks via
  gpsimd affine_select (zero out k>q entries after exp)
  PV: H^T[d, q] = [V|ones]^T @ E  -> heads rows 0:64 + sums row 64
  normalize via fast reciprocal of the sums row + gpsimd
  partition_broadcast + DVE multiplies
  out-proj: y[t, e] = H^T_tile^T @ WoT, accumulated over head pairs

All matmuls run in bf16 (1 cycle/row at any moving size); accumulation
is fp32 in PSUM.  RoPE is applied as rot = cos (.) pre + sin (.)
(P2 @ pre), where the head dim has been host-permuted to rotate-half
layout and P2 is the fixed swap-negate permutation folded into a
128x128 matmul.

The k-loop and projection loops are software-pipelined by one stage so
the PE never stalls on ACT (exp / psum evacuation) latency.
"""

import numpy as np
from contextlib import ExitStack

import concourse.bass as bass
from concourse import bacc
import concourse.tile as tile
import concourse.mybir as mybir
import ml_dtypes
from concourse.bass_utils import run_bass_kernel_spmd

F32 = mybir.dt.float32
BF16 = mybir.dt.bfloat16
AF = mybir.ActivationFunctionType
ALU = mybir.AluOpType

D_MODEL = 1024
NUM_HEADS = 16
THETA = 10000.0
B, T = 4, 2048
N_CORES = 8
PAIRS = 4             # head pairs per core (8 heads)
QC = 512              # q-chunk width
NQC = T // QC
KT = 128              # k-tile height
NKT = T // KT
XC = 256              # xt streaming chunk width (phase A)
NXC = T // XC
DMT = D_MODEL // 128  # 8 d_model k-tiles

_CACHE = {}


def _build_nc():
    nc = bacc.Bacc(None, target_bir_lowering=False)

    xt = nc.dram_tensor("xt", [D_MODEL, T], BF16, kind="ExternalInput")
    wq = nc.dram_tensor("wq", [D_MODEL, 512], BF16, kind="ExternalInput")
    wk = nc.dram_tensor("wk", [D_MODEL, 512], BF16, kind="ExternalInput")
    wv = nc.dram_tensor("wv", [D_MODEL, 512], BF16, kind="ExternalInput")
    wo = nc.dram_tensor("wo", [512, D_MODEL], BF16, kind="ExternalInput")
    cosd = nc.dram_tensor("cos", [128, T], F32, kind="ExternalInput")
    sind = nc.dram_tensor("sin", [128, T], F32, kind="ExternalInput")
    p2t = nc.dram_tensor("p2t", [128, 128], BF16, kind="ExternalInput")
    y = nc.dram_tensor("y", [T, D_MODEL], F32, kind="ExternalOutput")

    with tile.TileContext(nc) as tc, ExitStack() as ctx:
        constp = ctx.enter_context(tc.tile_pool(name="const", bufs=1))
        qkv_stack = ExitStack()
        qkp = qkv_stack.enter_context(tc.tile_pool(name="qk", bufs=1))
        vp = qkv_stack.enter_context(tc.tile_pool(name="v", bufs=1))

        cos_sb = constp.tile([128, T], F32)
        sin_sb = constp.tile([128, T], F32)
        p2_sb = constp.tile([128, 128], BF16)
        # wo lives in a long-lived pool so it can be prefetched in phase B
        wo_sb = constp.tile([128, PAIRS, D_MODEL], BF16)

        qt_t = [qkp.tile([128, T], BF16, name=f"qt{p}", tag=f"qt{p}")
                for p in range(PAIRS)]
        kt_t = [qkp.tile([128, T], BF16, name=f"kt{p}", tag=f"kt{p}")
                for p in range(PAIRS)]
        # V with interleaved ones cols: per k-tile, per pair:
        # [V_h0(64) | 1 | V_h1(64) | 1] = 130 cols
        v_sb = vp.tile([128, NKT, PAIRS * 130], BF16)
        v5 = v_sb.rearrange("p t (pr x) -> p t pr x", pr=PAIRS)
        v6 = v5.rearrange("p t pr (hl c) -> p t pr hl c", hl=2)
        nc.vector.memset(v6[:, :, :, :, 64:65], 1.0)

        # ---------------- Phase A: projections + RoPE ----------------
        with tc.tile_pool(name="wqkv", bufs=1) as wp, \
             tc.tile_pool(name="xtp", bufs=2) as xtp, \
             tc.tile_pool(name="sA", bufs=3) as sA, \
             tc.tile_pool(name="psA", bufs=2, space="PSUM") as psA, \
             tc.tile_pool(name="psV", bufs=2, space="PSUM") as psV:
            wq_sb = wp.tile([128, DMT, 512], BF16)
            wk_sb = wp.tile([128, DMT, 512], BF16)
            wv_sb = wp.tile([128, DMT, 512], BF16)
            xt_r = xt.rearrange("(a p) t -> p a t", p=128)
            wq_r = wq.rearrange("(a p) m -> p a m", p=128)
            wk_r = wk.rearrange("(a p) m -> p a m", p=128)
            wv_r = wv.rearrange("(a p) m -> p a m", p=128)
            # priority order: interleave wq + first x chunk per-dmt so the
            # first matmul group can start as soon as its slices land;
            # everything else trickles in behind them.
            pre_x = {}
            xs0 = xtp.tile([128, DMT, XC], BF16, tag="xt", name="xpre0")
            for dmt in range(DMT):
                nc.sync.dma_start(wq_sb[:, dmt, :], wq_r[:, dmt, :])
                nc.sync.dma_start(xs0[:, dmt, :], xt_r[:, dmt, 0:XC])
            pre_x[0] = xs0
            for dmt in range(DMT):
                nc.sync.dma_start(wk_sb[:, dmt, :], wk_r[:, dmt, :])
            xs1 = xtp.tile([128, DMT, XC], BF16, tag="xt", name="xpre1")
            nc.sync.dma_start(xs1, xt_r[:, :, XC:2 * XC])
            pre_x[1] = xs1
            nc.sync.dma_start(p2_sb, p2t[:])
            nc.sync.dma_start(cos_sb, cosd[:])
            nc.sync.dma_start(sin_sb, sind[:])
            for dmt in range(DMT):
                nc.sync.dma_start(wv_sb[:, dmt, :], wv_r[:, dmt, :])

            def emit_v(c, x_sb):
                # V for the two k-tiles of chunk c (deferred one chunk so
                # the wv DMA has time to land)
                for vt in range(2):
                    kti = c * 2 + vt
                    ps_v = psV.tile([128, 512], F32, tag="v", name="ps_v")
                    for dmt in range(DMT):
                        nc.tensor.matmul(
                            ps_v, x_sb[:, dmt, vt * 128:(vt + 1) * 128],
                            wv_sb[:, dmt, :],
                            start=(dmt == 0), stop=(dmt == DMT - 1))
                    src = ps_v.rearrange("p (pr hl c) -> p pr hl c",
                                         pr=PAIRS, hl=2)
                    nc.scalar.copy(v6[:, kti, :, :, 0:64], src)

            def rope_stage(st):
                pre, dst, cs = st
                ps_a = psA.tile([128, XC], F32, tag="alt", name="ps_a")
                nc.tensor.matmul(ps_a, p2_sb, pre, start=True, stop=True)
                t1 = sA.tile([128, XC], F32, tag="t1", name="t1")
                nc.vector.tensor_mul(t1, sin_sb[:, cs], ps_a)
                t2 = sA.tile([128, XC], F32, tag="t2", name="t2")
                nc.gpsimd.tensor_mul(t2, cos_sb[:, cs], pre)
                nc.vector.tensor_add(dst[:, cs], t1, t2)

            for c in range(NXC):
                cs = slice(c * XC, (c + 1) * XC)
                if c in pre_x:
                    x_sb = pre_x.pop(c)
                else:
                    x_sb = xtp.tile([128, DMT, XC], BF16, tag="xt",
                                    name="x_sb")
                    nc.sync.dma_start(x_sb, xt_r[:, :, cs])

                pend = None
                if c > 0:
                    emit_v(c - 1, pend_v)
                for (w_sb, dsts) in ((wq_sb, qt_t), (wk_sb, kt_t)):
                    for p in range(PAIRS):
                        ps = psA.tile([128, XC], F32, tag="proj", name="ps")
                        for dmt in range(DMT):
                            nc.tensor.matmul(
                                ps, w_sb[:, dmt, p * 128:(p + 1) * 128],
                                x_sb[:, dmt, :],
                                start=(dmt == 0), stop=(dmt == DMT - 1))
                        pre = sA.tile([128, XC], BF16, tag="pre", name="pre")
                        nc.scalar.copy(pre, ps)
                        if pend is not None:
                            rope_stage(pend)
                        pend = (pre, dsts[p], cs)

                rope_stage(pend)
                pend_v = x_sb
            emit_v(NXC - 1, pend_v)

        # ---------------- Phase B: attention ----------------
        hp = ctx.enter_context(tc.tile_pool(name="hp", bufs=1, side="right"))
        h_t = [hp.tile([128, T], BF16, name=f"h{p}", tag=f"h{p}")
               for p in range(PAIRS)]
        with tc.tile_pool(name="ep", bufs=4) as ep, \
             tc.tile_pool(name="h1p", bufs=2) as h1p, \
             tc.tile_pool(name="sB", bufs=2) as sB, \
             tc.tile_pool(name="rbp", bufs=2) as rbp, \
             tc.tile_pool(name="psS", bufs=2, space="PSUM") as psS, \
             tc.tile_pool(name="psH", bufs=2, space="PSUM") as psH:

            # prefetch the out-projection weights while the PE chews on
            # attention — the DMA engines are nearly idle in phase B
            nc.sync.dma_start(
                wo_sb, wo.rearrange("(a p) m -> p a m", p=128))

            def emit_pv(st):
                p, ps_h, e_sb, c0, k, last = st
                nc.tensor.matmul(ps_h[0:65, 0, c0:],
                                 v_sb[:, k, 130 * p:130 * p + 65],
                                 e_sb[:, 0, c0:],
                                 start=(k == 0), stop=last)
                nc.tensor.matmul(ps_h[0:65, 1, c0:],
                                 v_sb[:, k, 130 * p + 65:130 * p + 130],
                                 e_sb[:, 1, c0:],
                                 start=(k == 0), stop=last)

            def emit_norm(st):
                p, ps_h, qc, h1 = st
                r1 = sB.tile([1, 2, 512], F32, tag="r1", name="r1")
                nc.vector.reciprocal_approx_fast(out=r1, in_=ps_h[64:65, :, :])
                r64 = rbp.tile([64, 2, 512], F32, tag="r64", name="r64")
                nc.gpsimd.partition_broadcast(r64, r1)
                qs = slice(qc * QC, (qc + 1) * QC)
                nc.vector.tensor_mul(h_t[p][0:64, qs], ps_h[0:64, 0, :],
                                     r64[0:64, 0, :])
                nc.vector.tensor_mul(h1[0:64, :], ps_h[0:64, 1, :],
                                     r64[0:64, 1, :])
                # odd head rows into partitions 64:128 of the pair tile
                nc.sync.dma_start(h_t[p][64:128, qs], h1[0:64, :])

            pend_pv = []
            pend_norm = None
            for p in range(PAIRS):
                qt, kt = qt_t[p], kt_t[p]
                for qc in range(NQC):
                    nk = 4 * (qc + 1)
                    ps_h = psH.tile([128, 2, 512], F32, tag="pv", name="ps_h")
                    h1 = h1p.tile([64, QC], BF16, tag="h1", name="h1")
                    for k in range(nk):
                        m = k - 4 * qc
                        c0 = 128 * m if m >= 0 else 0
                        qs = slice(qc * QC + c0, (qc + 1) * QC)
                        ks = slice(k * KT, (k + 1) * KT)
                        ps_s = psS.tile([128, 2, 512], F32, tag="s",
                                        name="ps_s")
                        diag = m >= 0
                        nc.tensor.matmul(ps_s[:, 0, c0:], kt[0:64, ks],
                                         qt[0:64, qs], start=True,
                                         stop=True)
                        nc.tensor.matmul(ps_s[:, 1, c0:], kt[64:128, ks],
                                         qt[64:128, qs], start=True,
                                         stop=True)
                        e_sb = ep.tile([128, 2, 512], BF16, tag="e",
                                       name="e_sb")
                        nc.scalar.activation(e_sb[:, :, c0:], ps_s[:, :, c0:],
                                             AF.Exp, scale=0.125)
                        if diag:
                            # zero the strictly-upper-tri (k>q) entries of
                            # the diagonal block after exp
                            nc.gpsimd.affine_select(
                                e_sb[:, :, c0:c0 + 128],
                                e_sb[:, :, c0:c0 + 128],
                                pattern=[[0, 2], [1, 128]],
                                compare_op=ALU.is_ge,
                                fill=0.0, base=0, channel_multiplier=-1)
                        if len(pend_pv) >= 2:
                            emit_pv(pend_pv.pop(0))
                        pend_pv.append((p, ps_h, e_sb, c0, k, k == nk - 1))
                        # fire the deferred norm only once the previous
                        # q-chunk's last PV has drained out of pend_pv
                        if pend_norm is not None and k >= 2:
                            emit_norm(pend_norm)
                            pend_norm = None
                    pend_norm = (p, ps_h, qc, h1)
            for st in pend_pv:
                emit_pv(st)
            emit_norm(pend_norm)

        qkv_stack.close()

        # ---------------- Phase C: output projection ----------------
        with tc.tile_pool(name="yst", bufs=3) as yst, \
             tc.tile_pool(name="psY", bufs=4, space="PSUM") as psY:
            for tt in range(T // 128):
                ts_ = slice(tt * 128, (tt + 1) * 128)
                for ec in range(2):
                    es = slice(ec * 512, (ec + 1) * 512)
                    ps_y = psY.tile([128, 512], F32, tag="y", name="ps_y")
                    for p in range(PAIRS):
                        nc.tensor.matmul(ps_y, h_t[p][:, ts_],
                                         wo_sb[:, p, es],
                                         start=(p == 0),
                                         stop=(p == PAIRS - 1))
                    y_sb = yst.tile([128, 512], F32, tag="y", name="y_sb")
                    # alternate evacuation between ACT and DVE so neither
                    # becomes the phase C bottleneck
                    if ec == 0:
                        nc.scalar.copy(y_sb, ps_y)
                    else:
                        nc.vector.tensor_copy(y_sb, ps_y)
                    nc.sync.dma_start(y[ts_, es], y_sb)

    nc.compile()
    return nc


def _host_prep(in_features, token_positions, Wq, Wk, Wv, Wo):
    """Shard + pre-transpose + bf16-cast inputs for the 8 cores."""
    x = np.asarray(in_features, dtype=np.float32)
    pos = np.asarray(token_positions)
    Wq = np.asarray(Wq, dtype=np.float32)
    Wk = np.asarray(Wk, dtype=np.float32)
    Wv = np.asarray(Wv, dtype=np.float32)
    Wo = np.asarray(Wo, dtype=np.float32)

    # rotate-half permutation of each head's 64 dims: evens then odds
    perm = np.concatenate([np.arange(0, 64, 2), np.arange(1, 64, 2)])
    full_perm = (np.arange(NUM_HEADS)[:, None] * 64 + perm[None, :]).reshape(-1)
    Wq_p = Wq[full_perm, :]   # permute output rows (head dims)
    Wk_p = Wk[full_perm, :]

    # P2: alt = P2 @ pre (per 64-block: alt[i] = -pre[32+i], alt[32+i]=pre[i])
    p2 = np.zeros((128, 128), np.float32)
    for blk in (0, 64):
        for i in range(32):
            p2[blk + i, blk + 32 + i] = -1.0
            p2[blk + 32 + i, blk + i] = 1.0
    p2t = np.ascontiguousarray(p2.T).astype(ml_dtypes.bfloat16)

    inv_freq = 1.0 / (THETA ** (np.arange(32, dtype=np.float64) * 2.0 / 64))

    bf = ml_dtypes.bfloat16
    in_maps = []
    for core in range(N_CORES):
        b = core // 2
        g = core % 2
        hs = slice(g * 512, (g + 1) * 512)   # head-dim slice of d_model

        ang = pos[b].astype(np.float64)[None, :] * inv_freq[:, None]  # [32,T]
        cos64 = np.cos(ang).astype(np.float32)
        sin64 = np.sin(ang).astype(np.float32)
        cos128 = np.tile(np.concatenate([cos64, cos64], 0), (2, 1))   # [128,T]
        sin128 = np.tile(np.concatenate([sin64, sin64], 0), (2, 1))

        in_maps.append({
            "xt": np.ascontiguousarray(x[b].T).astype(bf),
            "wq": np.ascontiguousarray(Wq_p[hs, :].T).astype(bf),
            "wk": np.ascontiguousarray(Wk_p[hs, :].T).astype(bf),
            "wv": np.ascontiguousarray(Wv[hs, :].T).astype(bf),
            "wo": np.ascontiguousarray(Wo[:, hs].T).astype(bf),
            "cos": np.ascontiguousarray(cos128),
            "sin": np.ascontiguousarray(sin128),
            "p2t": p2t,
        })
    return in_maps


def kernel(**inputs):
    if "nc" not in _CACHE:
        _CACHE["nc"] = _build_nc()
    nc = _CACHE["nc"]
    in_maps = _host_prep(**inputs)
    res = run_bass_kernel_spmd(nc, in_maps, core_ids=list(range(N_CORES)))
    out = np.zeros((B, T, D_MODEL), np.float32)
    for core in range(N_CORES):
        out[core // 2] += res.results[core]["y"]
    return out
